# revision 6
# baseline (speedup 1.0000x reference)
"""Two-hop context attention (ContextAttentionDiffInput) Trainium2 Bass kernel.

Problem (B=4, D=512, L=2048, N=4096), fp32:
    K1 = elu(Wk1 @ H[b] + bk1)   (per batch, [D, L] "feature-major" layout)
    V1 = elu(H[b].T @ Wv1.T + bv1)  ([L, D] layout)
    (same for K2, V2)
    E1 = Qw @ K1          -> A1 = softmax_l(E1)
    C1 = A1 @ V1          -> E2 = C1 @ K2 -> A2 = softmax_l(E2)
    C2 = A2 @ V2
    returns (C2, A2)

Sharding: 8 cores = 4 batches x 2 halves of the N=4096 label dim.
Each core computes its batch's projections (replicated across the 2 cores
sharing a batch) and a 2048-label slice of hops 1+2.

Layout strategy (all matmuls fp32r, PSUM fp32):
  - hop1 scores E1T in [l, n] layout (direct from projection layouts, no
    transposes); softmax-over-l sum via ones-vector matmul (E1 in [-2.6, 2.6]
    so exp needs no max subtraction; verified against reference stats).
  - hop1 output kept unnormalized (U1T = V1.T @ exp(E1T), [e, n] layout);
    the 1/S1 row scale folds into hop2's exp(scale * x) per-partition scale.
  - hop2 scores in [n, l] layout (U1T slices are the lhsT directly), exp with
    accum_out giving the softmax denominator for free (E2 in [3, 28]: exp is
    safe unsubtracted in fp32).
  - A2 normalized on the way out; C2 accumulated from PE-transposed
    unnormalized A2 tiles, normalized by 1/S2 at the end.
"""

import numpy as np

import concourse.bass as bass
import concourse.mybir as mybir
import concourse.tile as tile
from concourse import bacc
from concourse.bass_utils import run_bass_kernel_spmd
from concourse.masks import make_identity

B, D, L, N = 4, 512, 2048, 4096
NCORES = 8
NSH = N // 2  # labels per core (N split in 2; batch split in 4)
NSLICES = NSH // 512  # hop1 processes 512 labels at a time
F32 = mybir.dt.float32
F32R = mybir.dt.float32r
EXP = mybir.ActivationFunctionType.Exp
ALU = mybir.AluOpType

_CACHE = {}
LAST_RESULTS = None
LAST_IN_MAPS = None


def _emit_elu(nc, pool, dst, z_ps, bias_ap=None, bias_tile=None):
    """dst = elu(z + bias), z in PSUM [128, 512].

    bias_ap: per-partition [128,1] AP (K-projections, bias along partitions)
    bias_tile: [128,512] broadcast tile (V-projections, bias along free dim)
    """
    t1 = pool.tile([128, 512], F32, tag="elu_t1", name="elu_t1")
    if bias_ap is not None:
        # t1 = exp(z + b); t3 = max(z + b, 0)
        nc.scalar.activation(t1[:, :], z_ps, EXP, bias=bias_ap)
        t3 = pool.tile([128, 512], F32, tag="elu_t3", name="elu_t3")
        nc.vector.tensor_scalar(
            out=t3[:, :], in0=z_ps, scalar1=bias_ap, scalar2=0.0,
            op0=ALU.add, op1=ALU.max,
        )
    else:
        zs = pool.tile([128, 512], F32, tag="elu_zs", name="elu_zs")
        nc.vector.tensor_tensor(out=zs[:, :], in0=z_ps, in1=bias_tile[:, :], op=ALU.add)
        nc.scalar.activation(t1[:, :], zs[:, :], EXP)
        t3 = pool.tile([128, 512], F32, tag="elu_t3", name="elu_t3")
        nc.vector.tensor_scalar(
            out=t3[:, :], in0=zs[:, :], scalar1=0.0, scalar2=None, op0=ALU.max,
        )
    t2 = pool.tile([128, 512], F32, tag="elu_t2", name="elu_t2")
    nc.vector.tensor_scalar(
        out=t2[:, :], in0=t1[:, :], scalar1=1.0, scalar2=0.0,
        op0=ALU.subtract, op1=ALU.min,
    )
    nc.vector.tensor_tensor(out=dst, in0=t3[:, :], in1=t2[:, :], op=ALU.add)


def _build():
    nc = bacc.Bacc("TRN2", target_bir_lowering=False, debug=False,
                   num_devices=NCORES)

    Hb = nc.dram_tensor("Hb", [D, L], F32R, kind="ExternalInput")
    WT = {w: nc.dram_tensor(w, [D, D], F32R, kind="ExternalInput")
          for w in ("Wk1T", "Wv1T", "Wk2T", "Wv2T")}
    BK = {b: nc.dram_tensor(b, [128, 4], F32, kind="ExternalInput")
          for b in ("bk1c", "bk2c")}
    BV = {b: nc.dram_tensor(b, [128, 512], F32, kind="ExternalInput")
          for b in ("bv1b", "bv2b")}
    QwTh = nc.dram_tensor("QwTh", [D, NSH], F32R, kind="ExternalInput")
    A2p = nc.dram_tensor("A2part", [NSH, L], F32, kind="ExternalOutput")
    C2p = nc.dram_tensor("C2part", [NSH, D], F32, kind="ExternalOutput")

    with tile.TileContext(nc) as tc:
        with (
            tc.tile_pool(name="const", bufs=1) as cpool,
            tc.tile_pool(name="persist", bufs=1) as ppool,
            tc.tile_pool(name="scores", bufs=3, space="PSUM") as ps_sc,
            tc.tile_pool(name="big4", bufs=1, space="PSUM") as ps_b4,
            tc.tile_pool(name="c2ps", bufs=1, space="PSUM") as ps_c2,
            tc.tile_pool(name="dram", bufs=1, space="DRAM") as dpool,
        ):
            ident_f = cpool.tile([128, 128], F32, tag="ident_f", name="ident_f")
            make_identity(nc, ident_f[:, :])
            ident = cpool.tile([128, 128], F32R, tag="ident", name="ident")
            nc.vector.tensor_copy(ident[:, :], ident_f[:, :])
            ones_f = cpool.tile([128, 1], F32, tag="ones_f", name="ones_f")
            nc.gpsimd.memset(ones_f[:, :], 1.0)
            ones = cpool.tile([128, 1], F32R, tag="ones", name="ones")
            nc.vector.tensor_copy(ones[:, :], ones_f[:, :])
            bk_sb = {}
            for name in ("bk1c", "bk2c"):
                t = cpool.tile([128, 4], F32, tag=name)
                nc.sync.dma_start(t[:, :], BK[name][:, :])
                bk_sb[name] = t
            bv_sb = {}
            for name in ("bv1b", "bv2b"):
                t = cpool.tile([128, 512], F32, tag=name)
                nc.sync.dma_start(t[:, :], BV[name][:, :])
                bv_sb[name] = t
            r1d = dpool.tile([1, NSH], F32, tag="r1d", name="r1d")

            # persistent projection outputs
            kt1 = [ppool.tile([128, L], F32R, tag=f"kt1_{ec}", name=f"kt1_{ec}") for ec in range(4)]
            kt2 = [ppool.tile([128, L], F32R, tag=f"kt2_{ec}", name=f"kt2_{ec}") for ec in range(4)]
            v1l = [ppool.tile([128, D], F32R, tag=f"v1l_{lt}", name=f"v1l_{lt}") for lt in range(16)]
            v2l = [ppool.tile([128, D], F32R, tag=f"v2l_{lt}", name=f"v2l_{lt}") for lt in range(16)]

            # ---------------- projections ----------------
            with tc.tile_pool(name="proj", bufs=2) as jpool:
                hb = []
                with tc.tile_pool(name="hbpool", bufs=1) as hpool:
                    for dc in range(4):
                        t = hpool.tile([128, L], F32R, tag=f"hb{dc}", name=f"hb{dc}")
                        nc.sync.dma_start(t[:, :], Hb[dc * 128:(dc + 1) * 128, :])
                        hb.append(t)

                    for wname, bname, dst, kind in (
                        ("Wk1T", "bk1c", kt1, "K"),
                        ("Wv1T", "bv1b", v1l, "V"),
                        ("Wk2T", "bk2c", kt2, "K"),
                        ("Wv2T", "bv2b", v2l, "V"),
                    ):
                        wt = []
                        for dc in range(4):
                            t = jpool.tile([128, D], F32R, tag=f"wt{dc}", name=f"wt{dc}")
                            nc.sync.dma_start(
                                t[:, :], WT[wname][dc * 128:(dc + 1) * 128, :])
                            wt.append(t)
                        if kind == "K":
                            # out [e-chunk, l]: lhsT = WT[:, e-chunk], rhs = H[d, l]
                            for ec in range(4):
                                for ls in range(4):
                                    ps = ps_sc.tile([128, 512], F32, tag="scores", name="scores")
                                    for dc in range(4):
                                        nc.tensor.matmul(
                                            ps[:, :],
                                            wt[dc][:, ec * 128:(ec + 1) * 128],
                                            hb[dc][:, ls * 512:(ls + 1) * 512],
                                            start=(dc == 0), stop=(dc == 3))
                                    _emit_elu(
                                        nc, jpool,
                                        dst[ec][:, ls * 512:(ls + 1) * 512],
                                        ps[:, :],
                                        bias_ap=bk_sb[bname][:, ec:ec + 1])
                        else:
                            # out [l-tile, e]: lhsT = H[d, l-tile], rhs = WT[d, e]
                            for lt in range(16):
                                ps = ps_sc.tile([128, 512], F32, tag="scores", name="scores")
                                for dc in range(4):
                                    nc.tensor.matmul(
                                        ps[:, :],
                                        hb[dc][:, lt * 128:(lt + 1) * 128],
                                        wt[dc][:, :],
                                        start=(dc == 0), stop=(dc == 3))
                                _emit_elu(nc, jpool, dst[lt][:, :], ps[:, :],
                                          bias_tile=bv_sb[bname])

            # ---------------- hops, one 512-label slice at a time ----------------
            hop_stack = tc.tile_pool(name="work", bufs=2)
            wpool = hop_stack.__enter__()
            p1_cm = tc.tile_pool(name="p1pool", bufs=3)
            p1pool = p1_cm.__enter__()
            a2ts_cm = tc.tile_pool(name="a2tspool", bufs=4)
            a2tspool = a2ts_cm.__enter__()
            for ns in range(NSLICES):
                qw = []
                for ec in range(4):
                    t = wpool.tile([128, 512], F32R, tag=f"qw{ec}", name=f"qw{ec}", bufs=1)
                    nc.sync.dma_start(
                        t[:, :],
                        QwTh[ec * 128:(ec + 1) * 128, ns * 512:(ns + 1) * 512])
                    qw.append(t)

                # hop1: P1 = exp(E1T) in [l, n]; U1T += V1.T @ P1; S1 += 1.T @ P1
                u1ps = ps_b4.tile([128, 2048], F32, tag="big4", name="big4")
                s1ps = ps_sc.tile([1, 512], F32, tag="scores", name="scores")
                p1_tiles = [None] * 16

                def consume_p1(j):
                    p1t = p1_tiles[j]
                    for ec in range(4):
                        nc.tensor.matmul(
                            u1ps[:, ec * 512:(ec + 1) * 512],
                            v1l[j][:, ec * 128:(ec + 1) * 128],
                            p1t[:, :], start=(j == 0), stop=(j == 15))
                    nc.tensor.matmul(s1ps[:, :], ones[:, :], p1t[:, :],
                                     start=(j == 0), stop=(j == 15))

                for lt in range(16):
                    e1 = ps_sc.tile([128, 512], F32, tag="scores", name="scores")
                    for ec in range(4):
                        nc.tensor.matmul(
                            e1[:, :],
                            kt1[ec][:, lt * 128:(lt + 1) * 128],
                            qw[ec][:, :], start=(ec == 0), stop=(ec == 3))
                    p1t = p1pool.tile([128, 512], F32R, tag="p1", name="p1")
                    nc.scalar.activation(p1t[:, :], e1[:, :], EXP)
                    p1_tiles[lt] = p1t
                    if lt >= 1:
                        consume_p1(lt - 1)
                consume_p1(15)

                u1sb = []
                for ec in range(4):
                    t = wpool.tile([128, 512], F32R, tag=f"u1sb{ec}", name=f"u1sb{ec}", bufs=1)
                    nc.vector.tensor_copy(t[:, :], u1ps[:, ec * 512:(ec + 1) * 512])
                    u1sb.append(t)
                r1row = wpool.tile([1, 512], F32, tag="r1row", name="r1row")
                nc.vector.reciprocal(r1row[:, :], s1ps[:, :])
                nc.sync.dma_start(r1d[0:1, ns * 512:(ns + 1) * 512], r1row[:, :])

                # hop2: per 128-label tile
                def emit_e2(nt):
                    g = ns * 4 + nt
                    r1c = wpool.tile([128, 1], F32, tag="r1c", name="r1c")
                    nc.sync.dma_start(
                        r1c[:, :],
                        r1d[0:1, g * 128:(g + 1) * 128].rearrange("o p -> p o"))
                    a2n = wpool.tile([128, L], F32R, tag="a2n", name="a2n")
                    s2p = []
                    for ls in range(4):
                        e2 = ps_sc.tile([128, 512], F32, tag="scores", name="scores")
                        for ec in range(4):
                            nc.tensor.matmul(
                                e2[:, :],
                                u1sb[ec][:, nt * 128:(nt + 1) * 128],
                                kt2[ec][:, ls * 512:(ls + 1) * 512],
                                start=(ec == 0), stop=(ec == 3))
                        sp = wpool.tile([128, 1], F32, tag=f"s2p{ls}", name=f"s2p{ls}")
                        nc.scalar.activation(
                            a2n[:, ls * 512:(ls + 1) * 512], e2[:, :], EXP,
                            scale=r1c[:, :], accum_out=sp[:, :])
                        s2p.append(sp)
                    return (nt, r1c, a2n, s2p)

                def emit_rest(state):
                    nt, r1c, a2n, s2p = state
                    g = ns * 4 + nt
                    s2a = wpool.tile([128, 1], F32, tag="s2a", name="s2a")
                    s2b = wpool.tile([128, 1], F32, tag="s2b", name="s2b")
                    nc.vector.tensor_tensor(out=s2a[:, :], in0=s2p[0][:, :],
                                            in1=s2p[1][:, :], op=ALU.add)
                    nc.vector.tensor_tensor(out=s2b[:, :], in0=s2p[2][:, :],
                                            in1=s2p[3][:, :], op=ALU.add)
                    nc.vector.tensor_tensor(out=s2a[:, :], in0=s2a[:, :],
                                            in1=s2b[:, :], op=ALU.add)
                    r2 = wpool.tile([128, 1], F32, tag="r2", name="r2")
                    nc.vector.reciprocal(r2[:, :], s2a[:, :])

                    # transposes of unnormalized A2 (packs of 4 128x128 blocks)
                    a2ts = []
                    for pk in range(4):
                        a2tp = ps_sc.tile([128, 4, 128], F32R, tag="scores", name="scores")
                        for c in range(4):
                            lc = pk * 4 + c
                            nc.tensor.transpose(
                                a2tp[:, c, :],
                                a2n[:, lc * 128:(lc + 1) * 128],
                                ident[:, :])
                        ts = a2tspool.tile([128, 4, 128], F32R, tag="a2ts", name="a2ts")
                        nc.vector.tensor_copy(ts[:, :, :], a2tp[:, :, :])
                        a2ts.append(ts)

                    # C2 accumulation
                    c2 = ps_c2.tile([128, 512], F32, tag="c2", name="c2")
                    for lc in range(16):
                        nc.tensor.matmul(
                            c2[:, :], a2ts[lc // 4][:, lc % 4, :], v2l[lc][:, :],
                            start=(lc == 0), stop=(lc == 15))

                    # outputs
                    a2o = wpool.tile([128, L], F32, tag="a2o", name="a2o", bufs=1)
                    for ls in range(4):
                        nc.vector.tensor_scalar(
                            out=a2o[:, ls * 512:(ls + 1) * 512],
                            in0=a2n[:, ls * 512:(ls + 1) * 512],
                            scalar1=r2[:, :], scalar2=None, op0=ALU.mult)
                    nc.sync.dma_start(A2p[g * 128:(g + 1) * 128, :], a2o[:, :])
                    c2s = wpool.tile([128, 512], F32, tag="c2s", name="c2s")
                    nc.vector.tensor_scalar(
                        out=c2s[:, :], in0=c2[:, :],
                        scalar1=r2[:, :], scalar2=None, op0=ALU.mult)
                    nc.sync.dma_start(C2p[g * 128:(g + 1) * 128, :], c2s[:, :])

                state = emit_e2(0)
                for nt in range(4):
                    nxt = emit_e2(nt + 1) if nt < 3 else None
                    emit_rest(state)
                    state = nxt

            a2ts_cm.__exit__(None, None, None)
            p1_cm.__exit__(None, None, None)
            hop_stack.__exit__(None, None, None)

    nc.compile()
    return nc


def _get_nc():
    if "nc" not in _CACHE:
        _CACHE["nc"] = _build()
    return _CACHE["nc"]


def kernel(H, Wk1, bk1, Wv1, bv1, Wk2, bk2, Wv2, bv2, Qw):
    global LAST_RESULTS
    H = np.asarray(H, dtype=np.float32)
    Qw = np.asarray(Qw, dtype=np.float32)
    nc = _get_nc()

    base = {}
    for name, w in (("Wk1T", Wk1), ("Wv1T", Wv1), ("Wk2T", Wk2), ("Wv2T", Wv2)):
        base[name] = np.ascontiguousarray(np.asarray(w, np.float32).T)
    for name, b in (("bk1c", bk1), ("bk2c", bk2)):
        base[name] = np.ascontiguousarray(
            np.asarray(b, np.float32).reshape(4, 128).T)
    for name, b in (("bv1b", bv1), ("bv2b", bv2)):
        base[name] = np.ascontiguousarray(
            np.broadcast_to(np.asarray(b, np.float32), (128, D)))

    in_maps = []
    for c in range(NCORES):
        b, nh = c // 2, c % 2
        m = dict(base)
        m["Hb"] = np.ascontiguousarray(H[b])
        m["QwTh"] = np.ascontiguousarray(Qw[nh * NSH:(nh + 1) * NSH].T)
        in_maps.append(m)

    globals()['LAST_IN_MAPS'] = in_maps
    res = run_bass_kernel_spmd(nc, in_maps, core_ids=list(range(NCORES)))
    LAST_RESULTS = res

    C2 = np.empty((B, N, D), np.float32)
    A2 = np.empty((B, N, L), np.float32)
    for c in range(NCORES):
        b, nh = c // 2, c % 2
        A2[b, nh * NSH:(nh + 1) * NSH] = res.results[c]["A2part"]
        C2[b, nh * NSH:(nh + 1) * NSH] = res.results[c]["C2part"]
    return C2, A2


# revision 8
# speedup vs baseline: 4.2700x; 4.2700x over previous
"""Two-hop context attention (ContextAttentionDiffInput) Trainium2 Bass kernel.

Problem (B=4, D=512, L=2048, N=4096), fp32:
    K1 = elu(Wk1 @ H[b] + bk1)   (per batch, [D, L] "feature-major" layout)
    V1 = elu(H[b].T @ Wv1.T + bv1)  ([L, D] layout)
    (same for K2, V2)
    E1 = Qw @ K1          -> A1 = softmax_l(E1)
    C1 = A1 @ V1          -> E2 = C1 @ K2 -> A2 = softmax_l(E2)
    C2 = A2 @ V2
    returns (C2, A2)

Sharding: 8 cores = 4 batches x 2 halves of the N=4096 label dim.
Each core computes its batch's projections (replicated across the 2 cores
sharing a batch) and a 2048-label slice of hops 1+2.

Layout strategy (all matmuls fp32r, PSUM fp32):
  - hop1 scores E1T in [l, n] layout (direct from projection layouts, no
    transposes); softmax-over-l sum via ones-vector matmul (E1 in [-2.6, 2.6]
    so exp needs no max subtraction; verified against reference stats).
  - hop1 output kept unnormalized (U1T = V1.T @ exp(E1T), [e, n] layout);
    the 1/S1 row scale folds into hop2's exp(scale * x) per-partition scale.
  - hop2 scores in [n, l] layout (U1T slices are the lhsT directly), exp with
    accum_out giving the softmax denominator for free (E2 in [3, 28]: exp is
    safe unsubtracted in fp32).
  - A2 normalized on the way out; C2 accumulated from PE-transposed
    unnormalized A2 tiles, normalized by 1/S2 at the end.
"""

import numpy as np

import concourse.bass as bass
import concourse.mybir as mybir
import concourse.tile as tile
from concourse import bacc
from concourse.bass_utils import run_bass_kernel_spmd
from concourse.masks import make_identity

B, D, L, N = 4, 512, 2048, 4096
NCORES = 8
NSH = N // 2  # labels per core (N split in 2; batch split in 4)
NSLICES = NSH // 512  # hop1 processes 512 labels at a time
F32 = mybir.dt.float32
F32R = mybir.dt.float32r
EXP = mybir.ActivationFunctionType.Exp
ALU = mybir.AluOpType

_CACHE = {}
LAST_RESULTS = None
LAST_IN_MAPS = None


def _emit_elu(nc, pool, dst, z_ps, bias_ap=None, bias_tile=None):
    """dst = elu(z + bias), z in PSUM [128, 512].

    bias_ap: per-partition [128,1] AP (K-projections, bias along partitions)
    bias_tile: [128,512] broadcast tile (V-projections, bias along free dim)
    """
    t1 = pool.tile([128, 512], F32, tag="elu_t1", name="elu_t1")
    if bias_ap is not None:
        # t1 = exp(z + b); t3 = max(z + b, 0)
        nc.scalar.activation(t1[:, :], z_ps, EXP, bias=bias_ap)
        t3 = pool.tile([128, 512], F32, tag="elu_t3", name="elu_t3")
        nc.vector.tensor_scalar(
            out=t3[:, :], in0=z_ps, scalar1=bias_ap, scalar2=0.0,
            op0=ALU.add, op1=ALU.max,
        )
    else:
        zs = pool.tile([128, 512], F32, tag="elu_zs", name="elu_zs")
        nc.vector.tensor_tensor(out=zs[:, :], in0=z_ps, in1=bias_tile[:, :], op=ALU.add)
        nc.scalar.activation(t1[:, :], zs[:, :], EXP)
        t3 = pool.tile([128, 512], F32, tag="elu_t3", name="elu_t3")
        nc.vector.tensor_scalar(
            out=t3[:, :], in0=zs[:, :], scalar1=0.0, scalar2=None, op0=ALU.max,
        )
    t2 = pool.tile([128, 512], F32, tag="elu_t2", name="elu_t2")
    nc.vector.tensor_scalar(
        out=t2[:, :], in0=t1[:, :], scalar1=1.0, scalar2=0.0,
        op0=ALU.subtract, op1=ALU.min,
    )
    nc.vector.tensor_tensor(out=dst, in0=t3[:, :], in1=t2[:, :], op=ALU.add)


def _build(reps=1):
    nc = bacc.Bacc("TRN2", target_bir_lowering=False, debug=False,
                   num_devices=NCORES)

    Hb = nc.dram_tensor("Hb", [D, L], F32R, kind="ExternalInput")
    WT = {w: nc.dram_tensor(w, [D, D], F32R, kind="ExternalInput")
          for w in ("Wk1T", "Wv1T", "Wk2T", "Wv2T")}
    BK = {b: nc.dram_tensor(b, [128, 4], F32, kind="ExternalInput")
          for b in ("bk1c", "bk2c")}
    BV = {b: nc.dram_tensor(b, [128, 512], F32, kind="ExternalInput")
          for b in ("bv1b", "bv2b")}
    QwTh = nc.dram_tensor("QwTh", [D, NSH], F32R, kind="ExternalInput")
    A2p = nc.dram_tensor("A2part", [NSH, L], F32, kind="ExternalOutput")
    C2p = nc.dram_tensor("C2part", [NSH, D], F32, kind="ExternalOutput")

    with tile.TileContext(nc) as tc:
        import contextlib
        rep_loop = tc.For_i(0, reps, 1) if reps > 1 else contextlib.nullcontext()
        with (
            rep_loop,
            tc.tile_pool(name="const", bufs=1) as cpool,
            tc.tile_pool(name="persist", bufs=1) as ppool,
            tc.tile_pool(name="scores", bufs=3, space="PSUM") as ps_sc,
            tc.tile_pool(name="big4", bufs=1, space="PSUM") as ps_b4,
            tc.tile_pool(name="c2ps", bufs=1, space="PSUM") as ps_c2,
            tc.tile_pool(name="dram", bufs=1, space="DRAM") as dpool,
        ):
            ident_f = cpool.tile([128, 128], F32, tag="ident_f", name="ident_f")
            make_identity(nc, ident_f[:, :])
            ident = cpool.tile([128, 128], F32R, tag="ident", name="ident")
            nc.vector.tensor_copy(ident[:, :], ident_f[:, :])
            ones_f = cpool.tile([128, 1], F32, tag="ones_f", name="ones_f")
            nc.gpsimd.memset(ones_f[:, :], 1.0)
            ones = cpool.tile([128, 1], F32R, tag="ones", name="ones")
            nc.vector.tensor_copy(ones[:, :], ones_f[:, :])
            bk_sb = {}
            for name in ("bk1c", "bk2c"):
                t = cpool.tile([128, 4], F32, tag=name)
                nc.sync.dma_start(t[:, :], BK[name][:, :])
                bk_sb[name] = t
            bv_sb = {}
            for name in ("bv1b", "bv2b"):
                t = cpool.tile([128, 512], F32, tag=name)
                nc.sync.dma_start(t[:, :], BV[name][:, :])
                bv_sb[name] = t
            r1d = dpool.tile([1, NSH], F32, tag="r1d", name="r1d")

            # persistent projection outputs
            kt1 = [ppool.tile([128, L], F32R, tag=f"kt1_{ec}", name=f"kt1_{ec}") for ec in range(4)]
            kt2 = [ppool.tile([128, L], F32R, tag=f"kt2_{ec}", name=f"kt2_{ec}") for ec in range(4)]
            v1l = [ppool.tile([128, D], F32R, tag=f"v1l_{lt}", name=f"v1l_{lt}") for lt in range(16)]
            v2l = [ppool.tile([128, D], F32R, tag=f"v2l_{lt}", name=f"v2l_{lt}") for lt in range(16)]

            # ---------------- projections ----------------
            with tc.tile_pool(name="proj", bufs=2) as jpool:
                hb = []
                with tc.tile_pool(name="hbpool", bufs=1) as hpool:
                    for dc in range(4):
                        t = hpool.tile([128, L], F32R, tag=f"hb{dc}", name=f"hb{dc}")
                        nc.sync.dma_start(t[:, :], Hb[dc * 128:(dc + 1) * 128, :])
                        hb.append(t)

                    for wname, bname, dst, kind in (
                        ("Wk1T", "bk1c", kt1, "K"),
                        ("Wv1T", "bv1b", v1l, "V"),
                        ("Wk2T", "bk2c", kt2, "K"),
                        ("Wv2T", "bv2b", v2l, "V"),
                    ):
                        wt = []
                        for dc in range(4):
                            t = jpool.tile([128, D], F32R, tag=f"wt{dc}", name=f"wt{dc}")
                            nc.sync.dma_start(
                                t[:, :], WT[wname][dc * 128:(dc + 1) * 128, :])
                            wt.append(t)
                        if kind == "K":
                            # out [e-chunk, l]: lhsT = WT[:, e-chunk], rhs = H[d, l]
                            for ec in range(4):
                                for ls in range(4):
                                    ps = ps_sc.tile([128, 512], F32, tag="scores", name="scores")
                                    for dc in range(4):
                                        nc.tensor.matmul(
                                            ps[:, :],
                                            wt[dc][:, ec * 128:(ec + 1) * 128],
                                            hb[dc][:, ls * 512:(ls + 1) * 512],
                                            start=(dc == 0), stop=(dc == 3))
                                    _emit_elu(
                                        nc, jpool,
                                        dst[ec][:, ls * 512:(ls + 1) * 512],
                                        ps[:, :],
                                        bias_ap=bk_sb[bname][:, ec:ec + 1])
                        else:
                            # out [l-tile, e]: lhsT = H[d, l-tile], rhs = WT[d, e]
                            for lt in range(16):
                                ps = ps_sc.tile([128, 512], F32, tag="scores", name="scores")
                                for dc in range(4):
                                    nc.tensor.matmul(
                                        ps[:, :],
                                        hb[dc][:, lt * 128:(lt + 1) * 128],
                                        wt[dc][:, :],
                                        start=(dc == 0), stop=(dc == 3))
                                _emit_elu(nc, jpool, dst[lt][:, :], ps[:, :],
                                          bias_tile=bv_sb[bname])

            # ---------------- hops, one 512-label slice at a time ----------------
            hop_stack = tc.tile_pool(name="work", bufs=2)
            wpool = hop_stack.__enter__()
            p1_cm = tc.tile_pool(name="p1pool", bufs=3)
            p1pool = p1_cm.__enter__()
            a2ts_cm = tc.tile_pool(name="a2tspool", bufs=4)
            a2tspool = a2ts_cm.__enter__()
            for ns in range(NSLICES):
                qw = []
                for ec in range(4):
                    t = wpool.tile([128, 512], F32R, tag=f"qw{ec}", name=f"qw{ec}", bufs=1)
                    nc.sync.dma_start(
                        t[:, :],
                        QwTh[ec * 128:(ec + 1) * 128, ns * 512:(ns + 1) * 512])
                    qw.append(t)

                # hop1: P1 = exp(E1T) in [l, n]; U1T += V1.T @ P1; S1 += 1.T @ P1
                u1ps = ps_b4.tile([128, 2048], F32, tag="big4", name="big4")
                s1ps = ps_sc.tile([1, 512], F32, tag="scores", name="scores")
                p1_tiles = [None] * 16

                def consume_p1(j):
                    p1t = p1_tiles[j]
                    for ec in range(4):
                        nc.tensor.matmul(
                            u1ps[:, ec * 512:(ec + 1) * 512],
                            v1l[j][:, ec * 128:(ec + 1) * 128],
                            p1t[:, :], start=(j == 0), stop=(j == 15))
                    nc.tensor.matmul(s1ps[:, :], ones[:, :], p1t[:, :],
                                     start=(j == 0), stop=(j == 15))

                for lt in range(16):
                    e1 = ps_sc.tile([128, 512], F32, tag="scores", name="scores")
                    for ec in range(4):
                        nc.tensor.matmul(
                            e1[:, :],
                            kt1[ec][:, lt * 128:(lt + 1) * 128],
                            qw[ec][:, :], start=(ec == 0), stop=(ec == 3))
                    p1t = p1pool.tile([128, 512], F32R, tag="p1", name="p1")
                    nc.scalar.activation(p1t[:, :], e1[:, :], EXP)
                    p1_tiles[lt] = p1t
                    if lt >= 1:
                        consume_p1(lt - 1)
                consume_p1(15)

                u1sb = []
                for ec in range(4):
                    t = wpool.tile([128, 512], F32R, tag=f"u1sb{ec}", name=f"u1sb{ec}", bufs=1)
                    nc.vector.tensor_copy(t[:, :], u1ps[:, ec * 512:(ec + 1) * 512])
                    u1sb.append(t)
                r1row = wpool.tile([1, 512], F32, tag="r1row", name="r1row")
                nc.vector.reciprocal(r1row[:, :], s1ps[:, :])
                nc.sync.dma_start(r1d[0:1, ns * 512:(ns + 1) * 512], r1row[:, :])

                # hop2: per 128-label tile
                def emit_e2(nt):
                    g = ns * 4 + nt
                    r1c = wpool.tile([128, 1], F32, tag="r1c", name="r1c")
                    nc.sync.dma_start(
                        r1c[:, :],
                        r1d[0:1, g * 128:(g + 1) * 128].rearrange("o p -> p o"))
                    a2n = wpool.tile([128, L], F32R, tag="a2n", name="a2n")
                    s2p = []
                    for ls in range(4):
                        e2 = ps_sc.tile([128, 512], F32, tag="scores", name="scores")
                        for ec in range(4):
                            nc.tensor.matmul(
                                e2[:, :],
                                u1sb[ec][:, nt * 128:(nt + 1) * 128],
                                kt2[ec][:, ls * 512:(ls + 1) * 512],
                                start=(ec == 0), stop=(ec == 3))
                        sp = wpool.tile([128, 1], F32, tag=f"s2p{ls}", name=f"s2p{ls}")
                        nc.scalar.activation(
                            a2n[:, ls * 512:(ls + 1) * 512], e2[:, :], EXP,
                            scale=r1c[:, :], accum_out=sp[:, :])
                        s2p.append(sp)
                    return (nt, r1c, a2n, s2p)

                def emit_rest(state):
                    nt, r1c, a2n, s2p = state
                    g = ns * 4 + nt
                    s2a = wpool.tile([128, 1], F32, tag="s2a", name="s2a")
                    s2b = wpool.tile([128, 1], F32, tag="s2b", name="s2b")
                    nc.vector.tensor_tensor(out=s2a[:, :], in0=s2p[0][:, :],
                                            in1=s2p[1][:, :], op=ALU.add)
                    nc.vector.tensor_tensor(out=s2b[:, :], in0=s2p[2][:, :],
                                            in1=s2p[3][:, :], op=ALU.add)
                    nc.vector.tensor_tensor(out=s2a[:, :], in0=s2a[:, :],
                                            in1=s2b[:, :], op=ALU.add)
                    r2 = wpool.tile([128, 1], F32, tag="r2", name="r2")
                    nc.vector.reciprocal(r2[:, :], s2a[:, :])

                    # transposes of unnormalized A2 (packs of 4 128x128 blocks)
                    a2ts = []
                    for pk in range(4):
                        a2tp = ps_sc.tile([128, 4, 128], F32R, tag="scores", name="scores")
                        for c in range(4):
                            lc = pk * 4 + c
                            nc.tensor.transpose(
                                a2tp[:, c, :],
                                a2n[:, lc * 128:(lc + 1) * 128],
                                ident[:, :])
                        ts = a2tspool.tile([128, 4, 128], F32R, tag="a2ts", name="a2ts")
                        nc.vector.tensor_copy(ts[:, :, :], a2tp[:, :, :])
                        a2ts.append(ts)

                    # C2 accumulation
                    c2 = ps_c2.tile([128, 512], F32, tag="c2", name="c2")
                    for lc in range(16):
                        nc.tensor.matmul(
                            c2[:, :], a2ts[lc // 4][:, lc % 4, :], v2l[lc][:, :],
                            start=(lc == 0), stop=(lc == 15))

                    # outputs
                    a2o = wpool.tile([128, L], F32, tag="a2o", name="a2o", bufs=1)
                    for ls in range(4):
                        nc.vector.tensor_scalar(
                            out=a2o[:, ls * 512:(ls + 1) * 512],
                            in0=a2n[:, ls * 512:(ls + 1) * 512],
                            scalar1=r2[:, :], scalar2=None, op0=ALU.mult)
                    nc.sync.dma_start(A2p[g * 128:(g + 1) * 128, :], a2o[:, :])
                    c2s = wpool.tile([128, 512], F32, tag="c2s", name="c2s")
                    nc.vector.tensor_scalar(
                        out=c2s[:, :], in0=c2[:, :],
                        scalar1=r2[:, :], scalar2=None, op0=ALU.mult)
                    nc.sync.dma_start(C2p[g * 128:(g + 1) * 128, :], c2s[:, :])

                state = emit_e2(0)
                for nt in range(4):
                    nxt = emit_e2(nt + 1) if nt < 3 else None
                    emit_rest(state)
                    state = nxt

            a2ts_cm.__exit__(None, None, None)
            p1_cm.__exit__(None, None, None)
            hop_stack.__exit__(None, None, None)

    nc.compile()
    return nc


def _get_nc(reps=1):
    key = f"nc{reps}"
    if key not in _CACHE:
        _CACHE[key] = _build(reps)
    return _CACHE[key]


def kernel(H, Wk1, bk1, Wv1, bv1, Wk2, bk2, Wv2, bv2, Qw):
    global LAST_RESULTS
    H = np.asarray(H, dtype=np.float32)
    Qw = np.asarray(Qw, dtype=np.float32)
    nc = _get_nc()

    base = {}
    for name, w in (("Wk1T", Wk1), ("Wv1T", Wv1), ("Wk2T", Wk2), ("Wv2T", Wv2)):
        base[name] = np.ascontiguousarray(np.asarray(w, np.float32).T)
    for name, b in (("bk1c", bk1), ("bk2c", bk2)):
        base[name] = np.ascontiguousarray(
            np.asarray(b, np.float32).reshape(4, 128).T)
    for name, b in (("bv1b", bv1), ("bv2b", bv2)):
        base[name] = np.ascontiguousarray(
            np.broadcast_to(np.asarray(b, np.float32), (128, D)))

    in_maps = []
    for c in range(NCORES):
        b, nh = c // 2, c % 2
        m = dict(base)
        m["Hb"] = np.ascontiguousarray(H[b])
        m["QwTh"] = np.ascontiguousarray(Qw[nh * NSH:(nh + 1) * NSH].T)
        in_maps.append(m)

    globals()['LAST_IN_MAPS'] = in_maps
    res = run_bass_kernel_spmd(nc, in_maps, core_ids=list(range(NCORES)))
    LAST_RESULTS = res

    C2 = np.empty((B, N, D), np.float32)
    A2 = np.empty((B, N, L), np.float32)
    for c in range(NCORES):
        b, nh = c // 2, c % 2
        A2[b, nh * NSH:(nh + 1) * NSH] = res.results[c]["A2part"]
        C2[b, nh * NSH:(nh + 1) * NSH] = res.results[c]["C2part"]
    return C2, A2
